# revision 6
# baseline (speedup 1.0000x reference)
"""Trainium2 Bass kernel for GNN cross-attention message passing.

Strategy (edge-parallel with destination-node sharding):
  - Host sorts edges by dst node; core c owns dst nodes [c*6250, (c+1)*6250).
  - Per core, edges are binned into 128-node blocks (padded to a fixed
    per-block capacity) so the segment softmax/sum is fully core-local.
  - Softmax max-subtraction is skipped: scores are clipped to [-5, 5] so
    exp() cannot overflow, and the 1e-16 denominator epsilon dominates the
    (identical up to ~1e-14 relative) normalization.
  - alpha normalization is deferred:  wV[n] = (sum_e w_e*(V+score)_e) / (sum_e w_e)
    which turns the per-edge division into one per-node multiply.
  - Device per edge-supertile (512 edges): gather KV[src], Q[dst] via
    indirect DMA; E = ea @ WE1 + bE1 via PE (bias via ones-row trick);
    score = K*Q*E on DVE (this is the wE output); per-head sums -> clip ->
    exp on ACT; one-hot(loc) built via is_equal; PSUM-accumulating matmul
    onehot.T @ [w*(V+score) | w] performs the 128-node segment sum.
  - No collectives needed at all.
"""

import math
import sys
from contextlib import ExitStack
from dataclasses import dataclass, field

import numpy as np

if "/opt/trn_rl_repo" not in sys.path:
    sys.path.insert(0, "/opt/trn_rl_repo")

import concourse.bass as bass
import concourse.mybir as mybir
import concourse.tile as tile
from concourse import bacc

P = 128
F32 = mybir.dt.float32
I32 = mybir.dt.int32

# ---------------------------------------------------------------------------
# configuration
# ---------------------------------------------------------------------------


@dataclass
class Cfg:
    n_nodes: int = 50000
    n_cores: int = 8
    in_dim: int = 64
    heads: int = 8
    dhead: int = 8
    clamp: float = 5.0
    st_edges: int = 512            # edges per supertile
    pre_group_tiles: int = 49      # node tiles per big xT load in precompute

    @property
    def hd(self):
        return self.heads * self.dhead

    @property
    def npc(self):  # nodes per core
        assert self.n_nodes % self.n_cores == 0
        return self.n_nodes // self.n_cores

    @property
    def nb(self):  # 128-node blocks per core
        return math.ceil(self.npc / P)

    @property
    def chunks_per_st(self):
        return self.st_edges // P

    @property
    def nt_pre(self):  # node tiles in the global Q/KV tables
        return math.ceil(self.n_nodes / P)

    @property
    def nodes_pad(self):
        return self.nt_pre * P


@dataclass
class Plan:
    cfg: Cfg
    bcap: int                      # edge capacity per 128-node block
    in_maps: list = field(default_factory=list)
    unshard: dict = field(default_factory=dict)

    @property
    def spb(self):  # supertiles per block
        return self.bcap // self.cfg.st_edges

    @property
    def nst(self):  # supertiles per core
        return self.cfg.nb * self.spb

    @property
    def ecap(self):  # padded edges per core
        return self.cfg.nb * self.bcap


# ---------------------------------------------------------------------------
# host-side sharding / layout prep
# ---------------------------------------------------------------------------


def host_prep(cfg: Cfg, x, edge_attr, WQ, bQ, WK, WV, WE1, bE1, edge_index,
              bcap: int | None = None) -> Plan:
    n_edges = edge_index.shape[1]
    src = np.ascontiguousarray(edge_index[0]).astype(np.int32)
    dst = np.ascontiguousarray(edge_index[1]).astype(np.int32)

    npc, nb = cfg.npc, cfg.nb
    core = dst // npc
    loc_node = dst - core * npc            # node index within core [0, npc)
    blk = loc_node // P                    # block within core [0, nb)
    loc = loc_node - blk * P               # node within block [0, 128)
    bin_id = core * nb + blk               # global bin [0, n_cores*nb)

    order = np.argsort(bin_id, kind="stable")
    counts = np.bincount(bin_id, minlength=cfg.n_cores * nb)
    max_count = int(counts.max())
    if bcap is None:
        bcap = max(cfg.st_edges,
                   math.ceil(max_count / cfg.st_edges) * cfg.st_edges)
    assert max_count <= bcap, (max_count, bcap)
    plan = Plan(cfg=cfg, bcap=bcap)

    # position of each (sorted) edge inside its bin
    bin_starts = np.zeros(cfg.n_cores * nb, np.int64)
    np.cumsum(counts[:-1], out=bin_starts[1:])
    within = np.arange(n_edges, dtype=np.int64) - bin_starts[bin_id[order]]
    # padded flat position within its core's edge array
    pos = (bin_id[order] % nb) * bcap + within
    ecore = bin_id[order] // nb            # core of each sorted edge

    ecap, nst, cps = plan.ecap, plan.nst, cfg.chunks_per_st
    jjn = bcap // P * 1                    # chunks per block
    njj = plan.spb * cps                   # == bcap // P

    # global tables input: xT with ones row, padded to nodes_pad
    xTa = np.zeros((cfg.in_dim + 1, cfg.nodes_pad), np.float32)
    xTa[:cfg.in_dim, :cfg.n_nodes] = np.ascontiguousarray(x.T)
    xTa[cfg.in_dim, :] = 1.0
    Wall = np.zeros((cfg.in_dim + 1, 3 * cfg.hd), np.float32)
    Wall[:cfg.in_dim, 0:cfg.hd] = WQ
    Wall[cfg.in_dim, 0:cfg.hd] = bQ
    Wall[:cfg.in_dim, cfg.hd:2 * cfg.hd] = WK
    Wall[:cfg.in_dim, 2 * cfg.hd:3 * cfg.hd] = WV
    WEb = np.zeros((cfg.in_dim + 1, cfg.hd), np.float32)
    WEb[:cfg.in_dim] = WE1
    WEb[cfg.in_dim] = bE1
    iota = np.arange(P, dtype=np.float32).reshape(1, P)

    unshard_rows = np.empty(n_edges, np.int64)   # row in core's wEs flat view
    unshard_core = ecore
    unshard_ids = order                          # original edge id per sorted edge

    for c in range(cfg.n_cores):
        m = ecore == c
        posc = pos[m]
        eidx = order[m]

        src_p = np.zeros(ecap, np.int32)
        dst_p = np.zeros(ecap, np.int32)
        loc_p = np.full(ecap, -1.0, np.float32)
        src_p[posc] = src[eidx]
        dst_p[posc] = dst[eidx]
        loc_p[posc] = loc[eidx].astype(np.float32)

        # [nb, P, njj] layout: edge q=b*bcap+jj*128+p  ->  [b, p, jj]
        def to_pjj(a):
            return np.ascontiguousarray(
                a.reshape(cfg.nb, njj, P).transpose(0, 2, 1))

        ea_p = np.zeros((ecap, cfg.in_dim), np.float32)
        ea_p[posc] = edge_attr[eidx]
        # [nst, in_dim+1, st_edges]; column j*128+p = edge st*512+j*128+p
        ea_t = np.empty((nst, cfg.in_dim + 1, cfg.st_edges), np.float32)
        ea_t[:, :cfg.in_dim, :] = (
            ea_p.reshape(nst, cps, P, cfg.in_dim)
            .transpose(0, 3, 1, 2).reshape(nst, cfg.in_dim, cfg.st_edges))
        ea_t[:, cfg.in_dim, :] = 1.0

        plan.in_maps.append({
            "xTa": xTa, "Wall": Wall, "WEb": WEb, "iota": iota,
            "ea": ea_t,
            "srcl": to_pjj(src_p), "dstl": to_pjj(dst_p), "locl": to_pjj(loc_p),
        })

        # wEs flat row for padded pos q: (q//512*128 + q%128)*cps + (q%512)//128
        q = posc
        unshard_rows[m] = ((q // cfg.st_edges * P + q % P) * cps
                           + (q % cfg.st_edges) // P)

    plan.unshard = dict(rows=unshard_rows, core=unshard_core, ids=unshard_ids,
                        n_edges=n_edges)
    return plan


# ---------------------------------------------------------------------------
# device program
# ---------------------------------------------------------------------------


def build_kernel_body(ctx: ExitStack, tc: tile.TileContext, outs, ins,
                      plan: Plan):
    cfg = plan.cfg
    nc = tc.nc
    hd, ind = cfg.hd, cfg.in_dim
    cps, spb, nst = cfg.chunks_per_st, plan.spb, plan.nst
    ste = cfg.st_edges
    clamp_hi = cfg.clamp * math.sqrt(cfg.dhead)
    inv_sqrt_d = 1.0 / math.sqrt(cfg.dhead)

    xTa, Wall, WEb, iota = ins["xTa"], ins["Wall"], ins["WEb"], ins["iota"]
    ea, srcl, dstl, locl = ins["ea"], ins["srcl"], ins["dstl"], ins["locl"]
    wEs, wVs = outs["wEs"], outs["wVs"]

    Qtab = nc.dram_tensor("Qtab", [cfg.nodes_pad, hd], F32, kind="Internal").ap()
    KVt = nc.dram_tensor("KVt", [cfg.nodes_pad, 2 * hd], F32, kind="Internal").ap()

    # --- weights / constants (resident) ---
    singles = ctx.enter_context(tc.tile_pool(name="singles", bufs=1))
    w_all = singles.tile([ind + 1, 3 * hd], F32)
    nc.sync.dma_start(out=w_all, in_=Wall)
    w_eb = singles.tile([ind + 1, hd], F32)
    nc.sync.dma_start(out=w_eb, in_=WEb)
    iota_t = singles.tile([P, P], F32)
    nc.gpsimd.dma_start(
        out=iota_t,
        in_=bass.AP(tensor=iota.tensor, offset=iota.offset,
                    ap=[[0, P], [1, P]]))

    # --- phase 1: Q/K/V node tables ---
    with tc.tile_pool(name="pre_big", bufs=2) as pre_big, \
         tc.tile_pool(name="pre_ps", bufs=4, space="PSUM") as pre_ps, \
         tc.tile_pool(name="pre_sb", bufs=4) as pre_sb:
        g_tiles = cfg.pre_group_tiles
        n_groups = math.ceil(cfg.nt_pre / g_tiles)
        for g in range(n_groups):
            t0 = g * g_tiles
            t1 = min(t0 + g_tiles, cfg.nt_pre)
            xbig = pre_big.tile([ind + 1, (t1 - t0) * P], F32)
            nc.sync.dma_start(out=xbig, in_=xTa[:, t0 * P:t1 * P])
            for k in range(t1 - t0):
                ps = pre_ps.tile([P, 3 * hd], F32, space="PSUM")
                nc.tensor.matmul(out=ps[:, :],
                                 lhsT=xbig[:, k * P:(k + 1) * P],
                                 rhs=w_all[:, :], start=True, stop=True)
                sb = pre_sb.tile([P, 3 * hd], F32)
                eng = nc.vector if k % 2 == 0 else nc.scalar
                if eng is nc.vector:
                    nc.vector.tensor_copy(out=sb[:, :], in_=ps[:, :])
                else:
                    nc.scalar.copy(out=sb[:, :], in_=ps[:, :])
                t = t0 + k
                nc.sync.dma_start(out=Qtab[t * P:(t + 1) * P, :],
                                  in_=sb[:, 0:hd])
                nc.sync.dma_start(out=KVt[t * P:(t + 1) * P, :],
                                  in_=sb[:, hd:3 * hd])

    # --- phase 2: edge processing ---
    idxp = ctx.enter_context(tc.tile_pool(name="idxp", bufs=2))
    eap = ctx.enter_context(tc.tile_pool(name="eap", bufs=3))
    gat = ctx.enter_context(tc.tile_pool(name="gat", bufs=3))
    eps_p = ctx.enter_context(tc.tile_pool(name="eps", bufs=3, space="PSUM"))
    accp = ctx.enter_context(tc.tile_pool(name="accp", bufs=2, space="PSUM"))
    work = ctx.enter_context(tc.tile_pool(name="work", bufs=3))
    outp = ctx.enter_context(tc.tile_pool(name="outp", bufs=3))

    njj = plan.bcap // P

    for b in range(cfg.nb):
        srcb = idxp.tile([P, njj], I32, tag="srcb")
        dstb = idxp.tile([P, njj], I32, tag="dstb")
        locb = idxp.tile([P, njj], F32, tag="locb")
        nc.sync.dma_start(out=srcb, in_=srcl[b])
        nc.sync.dma_start(out=dstb, in_=dstl[b])
        nc.sync.dma_start(out=locb, in_=locl[b])

        acc = accp.tile([P, hd + cfg.heads], F32, space="PSUM", tag="acc")

        for s in range(spb):
            st = b * spb + s
            j0 = s * cps

            ea_t = eap.tile([ind + 1, ste], F32, tag="ea")
            nc.sync.dma_start(out=ea_t, in_=ea[st])

            # HW indirect DMA honors exactly one index per partition per
            # instruction -> one gather per 128-edge chunk.
            kv_f = gat.tile([P, cps * 2 * hd], F32, tag="kv")
            for j in range(cps):
                nc.gpsimd.indirect_dma_start(
                    out=kv_f[:, j * 2 * hd:(j + 1) * 2 * hd], out_offset=None,
                    in_=KVt[:, :],
                    in_offset=bass.IndirectOffsetOnAxis(
                        ap=srcb[:, j0 + j:j0 + j + 1], axis=0))
            q_f = gat.tile([P, cps * hd], F32, tag="qg")
            for j in range(cps):
                nc.gpsimd.indirect_dma_start(
                    out=q_f[:, j * hd:(j + 1) * hd], out_offset=None,
                    in_=Qtab[:, :],
                    in_offset=bass.IndirectOffsetOnAxis(
                        ap=dstb[:, j0 + j:j0 + j + 1], axis=0))
            kv_g = kv_f.rearrange("p (c r) -> p c r", r=2 * hd)
            q_g = q_f.rearrange("p (c r) -> p c r", r=hd)

            e_ps = eps_p.tile([P, cps, hd], F32, space="PSUM", tag="eps")
            for j in range(cps):
                nc.tensor.matmul(out=e_ps[:, j, :],
                                 lhsT=ea_t[:, j * P:(j + 1) * P],
                                 rhs=w_eb[:, :], start=True, stop=True)

            kq = work.tile([P, cps, hd], F32, tag="kq")
            nc.vector.tensor_tensor(out=kq[:, :, :], in0=kv_g[:, :, 0:hd],
                                    in1=q_g[:, :, :], op=mybir.AluOpType.mult)
            score = outp.tile([P, cps, hd], F32, tag="score")
            nc.vector.tensor_tensor(out=score[:, :, :], in0=kq[:, :, :],
                                    in1=e_ps[:, :, :], op=mybir.AluOpType.mult)
            nc.sync.dma_start(out=wEs[st], in_=score[:, :, :])

            sc = work.tile([P, cps, cfg.heads], F32, tag="sc")
            nc.vector.tensor_reduce(
                out=sc[:, :, :],
                in_=score.rearrange("p c (h d) -> p c h d", d=cfg.dhead),
                axis=mybir.AxisListType.X, op=mybir.AluOpType.add)
            nc.vector.tensor_scalar(
                out=sc[:, :, :], in0=sc[:, :, :],
                scalar1=clamp_hi, scalar2=-clamp_hi,
                op0=mybir.AluOpType.min, op1=mybir.AluOpType.max)

            contrib = work.tile([P, cps, hd + cfg.heads], F32, tag="contrib")
            w_ap = contrib[:, :, hd:hd + cfg.heads]
            nc.scalar.activation(out=w_ap, in_=sc[:, :, :],
                                 func=mybir.ActivationFunctionType.Exp,
                                 scale=inv_sqrt_d)

            vps = work.tile([P, cps, hd], F32, tag="vps")
            nc.vector.tensor_tensor(out=vps[:, :, :], in0=kv_g[:, :, hd:2 * hd],
                                    in1=score[:, :, :], op=mybir.AluOpType.add)
            w_b = bass.AP(tensor=w_ap.tensor, offset=w_ap.offset,
                          ap=[*w_ap.ap, [0, cfg.dhead]])
            nc.vector.tensor_tensor(
                out=contrib.rearrange("p c (h d) -> p c h d", d=cfg.dhead)[
                    :, :, 0:cfg.heads, :],
                in0=vps.rearrange("p c (h d) -> p c h d", d=cfg.dhead),
                in1=w_b, op=mybir.AluOpType.mult)

            oh = work.tile([P, cps, P], F32, tag="oh")
            loc_s = locb[:, j0:j0 + cps]
            loc_b = bass.AP(tensor=loc_s.tensor, offset=loc_s.offset,
                            ap=[*loc_s.ap, [0, P]])
            it_ap = iota_t[:, :]
            iota_b = bass.AP(tensor=it_ap.tensor, offset=it_ap.offset,
                             ap=[it_ap.ap[0], [0, cps], it_ap.ap[1]])
            nc.vector.tensor_tensor(out=oh[:, :, :], in0=loc_b, in1=iota_b,
                                    op=mybir.AluOpType.is_equal)

            for j in range(cps):
                nc.tensor.matmul(out=acc[:, :], lhsT=oh[:, j, :],
                                 rhs=contrib[:, j, :],
                                 start=(s == 0 and j == 0),
                                 stop=(s == spb - 1 and j == cps - 1))

        # --- block finalize: wV = U / (den + 1e-16) ---
        recip = outp.tile([P, cfg.heads], F32, tag="recip")
        nc.vector.tensor_scalar(out=recip, in0=acc[:, hd:hd + cfg.heads],
                                scalar1=1e-16, scalar2=None,
                                op0=mybir.AluOpType.add)
        nc.vector.reciprocal(out=recip, in_=recip)
        wv_t = outp.tile([P, hd], F32, tag="wv")
        r_b = bass.AP(tensor=recip.tensor, offset=recip.offset,
                      ap=[*recip.ap, [0, cfg.dhead]])
        nc.vector.tensor_tensor(
            out=wv_t.rearrange("p (h d) -> p h d", d=cfg.dhead),
            in0=acc.rearrange("p f -> p f")[:, 0:hd].rearrange(
                "p (h d) -> p h d", d=cfg.dhead),
            in1=r_b, op=mybir.AluOpType.mult)
        nc.sync.dma_start(out=wVs[b * P:(b + 1) * P, :], in_=wv_t)


def build_program(plan: Plan):
    cfg = plan.cfg
    nc = bacc.Bacc("TRN2", target_bir_lowering=False, debug=False,
                   enable_asserts=False)
    ins = {}
    for name, arr in plan.in_maps[0].items():
        ins[name] = nc.dram_tensor(name, list(arr.shape),
                                   mybir.dt.from_np(arr.dtype),
                                   kind="ExternalInput").ap()
    outs = {
        "wEs": nc.dram_tensor("wEs", [plan.nst, P, cfg.st_edges // P * cfg.hd],
                              F32, kind="ExternalOutput").ap(),
        "wVs": nc.dram_tensor("wVs", [cfg.nb * P, cfg.hd], F32,
                              kind="ExternalOutput").ap(),
    }
    with tile.TileContext(nc) as tc:
        with ExitStack() as ctx:
            build_kernel_body(ctx, tc, outs, ins, plan)
    nc.compile()
    return nc


# ---------------------------------------------------------------------------
# top-level entry
# ---------------------------------------------------------------------------


def unshard(plan: Plan, wEs_list, wVs_list):
    cfg = plan.cfg
    u = plan.unshard
    wE = np.empty((u["n_edges"], cfg.hd), np.float32)
    for c in range(cfg.n_cores):
        m = u["core"] == c
        flat = wEs_list[c].reshape(-1, cfg.hd)
        wE[u["ids"][m]] = flat[u["rows"][m]]
    wV = np.concatenate([wVs_list[c][:cfg.npc] for c in range(cfg.n_cores)])
    return wV.reshape(cfg.n_nodes, cfg.heads, cfg.dhead), wE


def kernel(x, edge_attr, WQ, bQ, WK, WV, WE1, bE1, edge_index):
    from concourse.bass_utils import run_bass_kernel_spmd

    cfg = Cfg()
    plan = host_prep(cfg, np.asarray(x, np.float32),
                     np.asarray(edge_attr, np.float32),
                     np.asarray(WQ, np.float32), np.asarray(bQ, np.float32),
                     np.asarray(WK, np.float32), np.asarray(WV, np.float32),
                     np.asarray(WE1, np.float32), np.asarray(bE1, np.float32),
                     np.asarray(edge_index))
    nc = build_program(plan)
    res = run_bass_kernel_spmd(nc, plan.in_maps,
                               core_ids=list(range(cfg.n_cores)))
    wEs = [r["wEs"] for r in res.results]
    wVs = [r["wVs"] for r in res.results]
    return unshard(plan, wEs, wVs)


# revision 23
# speedup vs baseline: 1.0200x; 1.0200x over previous
"""Trainium2 Bass kernel for GNN cross-attention message passing.

Strategy (edge-parallel with destination-node sharding):
  - Host sorts edges by (dst block, src-half); core c owns dst nodes
    [c*6250, (c+1)*6250).
  - Per core, edges are binned into 128-node dst blocks (padded to a fixed
    per-block capacity) so the segment softmax/sum is fully core-local.
  - Softmax max-subtraction is skipped: scores are clipped to [-5, 5] so
    exp() cannot overflow; the normalization is identical to ~1e-14 rel.
  - alpha normalization is deferred:  wV[n] = (sum_e w_e*(V+score)_e) / (sum_e w_e)
  - Node tables: K|V interleaved [N,128] split in two halves so rows are
    int16-addressable by the dma_gather custom SWDGE instruction; edges are
    additionally grouped by src-half so each supertile gathers from exactly
    one half. Q is per-core local (dst is core-local by construction).
  - Device per block (128 dst nodes): three dma_gathers fetch KV[src] and
    Q[dst] for the whole block; per 512-edge supertile: E = ea@WE1+bE1 on
    PE (bias via ones-row); score = K*Q*E on DVE (= wE output); per-head
    sums -> clip -> exp on ACT; V+score on GPSIMD; one-hot(loc) via
    is_equal; PSUM-accumulating matmul onehot.T @ [w*(V+score) | w] is the
    128-node segment sum. wV = U/(den+1e-16) per block.
  - No collectives needed at all.
"""

import math
import sys
from contextlib import ExitStack
from dataclasses import dataclass, field

import numpy as np

if "/opt/trn_rl_repo" not in sys.path:
    sys.path.insert(0, "/opt/trn_rl_repo")

import concourse.bass as bass
import concourse.mybir as mybir
import concourse.tile as tile
from concourse import bacc

P = 128
F32 = mybir.dt.float32
I16 = mybir.dt.int16

# ---------------------------------------------------------------------------
# configuration
# ---------------------------------------------------------------------------


@dataclass
class Cfg:
    n_nodes: int = 50000
    n_cores: int = 8
    in_dim: int = 64
    heads: int = 8
    dhead: int = 8
    clamp: float = 5.0
    st_edges: int = 512            # edges per supertile
    half_rows: int = 32768         # KV table split for int16 dma_gather idxs
    pre_group_tiles: int = 49      # node tiles per big xT load in precompute

    @property
    def hd(self):
        return self.heads * self.dhead

    @property
    def npc(self):  # nodes per core
        assert self.n_nodes % self.n_cores == 0
        return self.n_nodes // self.n_cores

    @property
    def nb(self):  # 128-node blocks per core
        return math.ceil(self.npc / P)

    @property
    def npc_pad(self):
        return self.nb * P

    @property
    def chunks_per_st(self):
        return self.st_edges // P

    @property
    def nt_pre(self):  # node tiles in the global KV tables
        return math.ceil(self.n_nodes / P)

    @property
    def nodes_pad(self):
        return self.nt_pre * P


@dataclass
class Plan:
    cfg: Cfg
    slo: int                       # supertiles per block gathering from KVlo
    shi: int                       # ... from KVhi
    in_maps: list = field(default_factory=list)
    unshard: dict = field(default_factory=dict)

    @property
    def spb(self):  # supertiles per block
        return self.slo + self.shi

    @property
    def bcap(self):
        return self.spb * self.cfg.st_edges

    @property
    def nst(self):  # supertiles per core
        return self.cfg.nb * self.spb

    @property
    def ecap(self):  # padded edges per core
        return self.cfg.nb * self.bcap


# ---------------------------------------------------------------------------
# host-side sharding / layout prep
# ---------------------------------------------------------------------------


def _wrap16(a, nb):
    """[nb, n] int array -> [nb, 128, n//16] int16, value i at [b, i%16, i//16],
    replicated across the 8 gpsimd partition groups."""
    n = a.shape[1]
    w = a.reshape(nb, n // 16, 16).transpose(0, 2, 1)
    return np.ascontiguousarray(np.tile(w, (1, 8, 1)).astype(np.int16))


def host_prep(cfg: Cfg, x, edge_attr, WQ, bQ, WK, WV, WE1, bE1, edge_index,
              caps: tuple[int, int] | None = None) -> Plan:
    n_edges = edge_index.shape[1]
    src = np.ascontiguousarray(edge_index[0]).astype(np.int32)
    dst = np.ascontiguousarray(edge_index[1]).astype(np.int32)

    npc, nb, ste = cfg.npc, cfg.nb, cfg.st_edges
    HALF = cfg.half_rows
    core = dst // npc
    loc_node = dst - core * npc
    blk = loc_node // P
    loc = loc_node - blk * P
    half = (src >= HALF).astype(np.int32)
    bin2 = (core * nb + blk) * 2 + half

    order = np.argsort(bin2, kind="stable")
    counts2 = np.bincount(bin2, minlength=cfg.n_cores * nb * 2)
    lo_max = int(counts2[0::2].max())
    hi_max = int(counts2[1::2].max())
    if caps is None:
        slo = max(1, math.ceil(lo_max / ste))
        shi = math.ceil(hi_max / ste)
    else:
        slo, shi = caps
    assert lo_max <= slo * ste and hi_max <= shi * ste
    plan = Plan(cfg=cfg, slo=slo, shi=shi)
    bcap, nst, ecap = plan.bcap, plan.nst, plan.ecap
    cps, spb, njj = cfg.chunks_per_st, plan.spb, plan.bcap // P
    nlo, nhi = slo * ste, shi * ste

    bin_starts = np.zeros(cfg.n_cores * nb * 2, np.int64)
    np.cumsum(counts2[:-1], out=bin_starts[1:])
    within = np.arange(n_edges, dtype=np.int64) - bin_starts[bin2[order]]
    ob = bin2[order]
    pos = (ob // 2 % nb) * bcap + (ob % 2) * nlo + within
    ecore = ob // (2 * nb)

    # global tables input: xT with ones row, padded to nodes_pad
    xTa = np.zeros((cfg.in_dim + 1, cfg.nodes_pad), np.float32)
    xTa[:cfg.in_dim, :cfg.n_nodes] = np.ascontiguousarray(x.T)
    xTa[cfg.in_dim, :] = 1.0
    Wall = np.zeros((cfg.in_dim + 1, 3 * cfg.hd), np.float32)
    Wall[:cfg.in_dim, 0:cfg.hd] = WQ
    Wall[cfg.in_dim, 0:cfg.hd] = bQ
    Wall[:cfg.in_dim, cfg.hd:2 * cfg.hd] = WK
    Wall[:cfg.in_dim, 2 * cfg.hd:3 * cfg.hd] = WV
    WEb = np.zeros((cfg.in_dim + 1, cfg.hd), np.float32)
    WEb[:cfg.in_dim] = WE1
    WEb[cfg.in_dim] = bE1
    iota = np.arange(P, dtype=np.float32).reshape(1, P)

    # region masks over a padded core array
    qq = np.arange(ecap, dtype=np.int64)
    is_hi_region = (qq % bcap) >= nlo

    unshard_rows = np.empty(n_edges, np.int64)
    for c in range(cfg.n_cores):
        m = ecore == c
        posc = pos[m]
        eidx = order[m]

        src_p = np.where(is_hi_region, HALF, 0).astype(np.int32)
        dst_p = np.full(ecap, c * npc, np.int32)
        loc_p = np.full(ecap, -1.0, np.float32)
        src_p[posc] = src[eidx]
        dst_p[posc] = dst[eidx]
        loc_p[posc] = loc[eidx].astype(np.float32)

        kv_local = src_p - HALF * is_hi_region.astype(np.int32)
        q_local = dst_p - c * npc
        assert kv_local.min() >= 0 and q_local.min() >= 0
        assert kv_local.max() < HALF and q_local.max() < cfg.npc_pad

        kvr = kv_local.reshape(nb, bcap)
        kvidx_lo = _wrap16(kvr[:, :nlo], nb)
        kvidx_hi = _wrap16(kvr[:, nlo:], nb)
        qidx = _wrap16(q_local.reshape(nb, bcap), nb)
        locl = np.ascontiguousarray(
            loc_p.reshape(nb, njj, P).transpose(0, 2, 1))

        ea_p = np.zeros((ecap, cfg.in_dim), np.float32)
        ea_p[posc] = edge_attr[eidx]
        # [nb, in_dim+1, bcap]; column jj*128+p = edge b*bcap+jj*128+p
        ea_t = np.empty((nb, cfg.in_dim + 1, bcap), np.float32)
        ea_t[:, :cfg.in_dim, :] = (
            ea_p.reshape(nb, njj, P, cfg.in_dim)
            .transpose(0, 3, 1, 2).reshape(nb, cfg.in_dim, bcap))
        ea_t[:, cfg.in_dim, :] = 1.0

        xTa2 = np.ascontiguousarray(xTa[:, c * npc:c * npc + cfg.npc_pad])

        plan.in_maps.append({
            "xTa": xTa, "xTa2": xTa2, "Wall": Wall, "WEb": WEb, "iota": iota,
            "ea": ea_t, "kvlo_i": kvidx_lo, "kvhi_i": kvidx_hi, "q_i": qidx,
            "locl": locl,
        })

        q = posc
        unshard_rows[m] = ((q // bcap * P + q % P) * njj + (q % bcap) // P)

    plan.unshard = dict(rows=unshard_rows, core=ecore, ids=order,
                        n_edges=n_edges)
    return plan


# ---------------------------------------------------------------------------
# device program
# ---------------------------------------------------------------------------


def build_kernel_body(ctx: ExitStack, tc: tile.TileContext, outs, ins,
                      plan: Plan, repeat: int = 1):
    cfg = plan.cfg
    nc = tc.nc
    hd, ind = cfg.hd, cfg.in_dim
    cps, spb, nst = cfg.chunks_per_st, plan.spb, plan.nst
    ste, njj = cfg.st_edges, plan.bcap // P
    slo, shi = plan.slo, plan.shi
    nlo, nhi = slo * ste, shi * ste
    HALF = cfg.half_rows
    clamp_hi = cfg.clamp * math.sqrt(cfg.dhead)
    inv_sqrt_d = 1.0 / math.sqrt(cfg.dhead)

    xTa, xTa2, Wall, WEb = ins["xTa"], ins["xTa2"], ins["Wall"], ins["WEb"]
    iota, ea = ins["iota"], ins["ea"]
    kvlo_i, kvhi_i, q_i, locl = (ins["kvlo_i"], ins["kvhi_i"], ins["q_i"],
                                 ins["locl"])
    wEs, wVs = outs["wEs"], outs["wVs"]

    n_hi_rows = max(P, cfg.nodes_pad - HALF)
    KVlo = nc.dram_tensor("KVlo", [min(HALF, cfg.nodes_pad), 2 * hd], F32,
                          kind="Internal").ap()
    KVhi = nc.dram_tensor("KVhi", [n_hi_rows, 2 * hd], F32,
                          kind="Internal").ap()
    Qt = nc.dram_tensor("Qt", [cfg.npc_pad, hd], F32, kind="Internal").ap()

    # --- weights / constants (resident) ---
    singles = ctx.enter_context(tc.tile_pool(name="singles", bufs=1))
    w_all = singles.tile([ind + 1, 3 * hd], F32)
    nc.sync.dma_start(out=w_all, in_=Wall)
    w_eb = singles.tile([ind + 1, hd], F32)
    nc.sync.dma_start(out=w_eb, in_=WEb)
    iota_t = singles.tile([P, P], F32)
    nc.gpsimd.dma_start(
        out=iota_t,
        in_=bass.AP(tensor=iota.tensor, offset=iota.offset,
                    ap=[[0, P], [1, P]]))

    # --- phase 1: K/V global tables + per-core Q table ---
    with tc.tile_pool(name="pre_big", bufs=2) as pre_big, \
         tc.tile_pool(name="pre_ps", bufs=4, space="PSUM") as pre_ps, \
         tc.tile_pool(name="pre_sb", bufs=4) as pre_sb:
        g_tiles = cfg.pre_group_tiles
        n_groups = math.ceil(cfg.nt_pre / g_tiles)
        half_t = HALF // P

        def staged_table_write(table, t0, nt, stage, width):
            """One batched DMA: stage [P, nt, width] -> table rows t0*P.."""
            dst = table[t0 * P:(t0 + nt) * P, :].rearrange(
                "(k p) d -> p k d", p=P)
            nc.sync.dma_start(out=dst, in_=stage[:, :nt, :])

        for g in range(n_groups):
            t0 = g * g_tiles
            t1 = min(t0 + g_tiles, cfg.nt_pre)
            xbig = pre_big.tile([ind + 1, g_tiles * P], F32, tag="xbig")
            nc.sync.dma_start(out=xbig[:, :(t1 - t0) * P],
                              in_=xTa[:, t0 * P:t1 * P])
            stage = pre_sb.tile([P, g_tiles, 2 * hd], F32, tag="stage", bufs=2)
            for k in range(t1 - t0):
                ps = pre_ps.tile([P, 2 * hd], F32, space="PSUM", tag="ps")
                nc.tensor.matmul(out=ps[:, :],
                                 lhsT=xbig[:, k * P:(k + 1) * P],
                                 rhs=w_all[:, hd:3 * hd], start=True,
                                 stop=True)
                if k % 2 == 0:
                    nc.vector.tensor_copy(out=stage[:, k, :], in_=ps[:, :])
                else:
                    nc.scalar.copy(out=stage[:, k, :], in_=ps[:, :])
            # batched write, split at the lo/hi table boundary
            nt = t1 - t0
            if t1 <= half_t:
                staged_table_write(KVlo, t0, nt, stage, 2 * hd)
            elif t0 >= half_t:
                staged_table_write(KVhi, t0 - half_t, nt, stage, 2 * hd)
            else:
                nsplit = half_t - t0
                staged_table_write(KVlo, t0, nsplit, stage, 2 * hd)
                dst = KVhi[0:(nt - nsplit) * P, :].rearrange(
                    "(k p) d -> p k d", p=P)
                nc.sync.dma_start(out=dst, in_=stage[:, nsplit:nt, :])
        # per-core Q table
        xb2 = pre_big.tile([ind + 1, cfg.npc_pad], F32, tag="xb2")
        nc.sync.dma_start(out=xb2, in_=xTa2)
        qstage = pre_sb.tile([P, cfg.nb, hd], F32, tag="qstage", bufs=1)
        for k in range(cfg.nb):
            psq = pre_ps.tile([P, hd], F32, space="PSUM", tag="psq")
            nc.tensor.matmul(out=psq[:, :], lhsT=xb2[:, k * P:(k + 1) * P],
                             rhs=w_all[:, 0:hd], start=True, stop=True)
            if k % 2 == 0:
                nc.vector.tensor_copy(out=qstage[:, k, :], in_=psq[:, :])
            else:
                nc.scalar.copy(out=qstage[:, k, :], in_=psq[:, :])
        nc.sync.dma_start(
            out=Qt[:, :].rearrange("(k p) d -> p k d", p=P), in_=qstage)

    # --- phase 2: edge processing ---
    idxp = ctx.enter_context(tc.tile_pool(name="idxp", bufs=2))
    eap = ctx.enter_context(tc.tile_pool(name="eap", bufs=4))
    gat = ctx.enter_context(tc.tile_pool(name="gat", bufs=2))
    eps_p = ctx.enter_context(tc.tile_pool(name="eps", bufs=3, space="PSUM"))
    accp = ctx.enter_context(tc.tile_pool(name="accp", bufs=2, space="PSUM"))
    work = ctx.enter_context(tc.tile_pool(name="work", bufs=3))
    outp = ctx.enter_context(tc.tile_pool(name="outp", bufs=3))

    for _rep in range(repeat):
      for b in range(cfg.nb):
        locb = idxp.tile([P, njj], F32, tag="locb")
        nc.sync.dma_start(out=locb, in_=locl[b])
        kvil = idxp.tile([P, nlo // 16], I16, tag="kvil")
        nc.sync.dma_start(out=kvil, in_=kvlo_i[b])
        if nhi:
            kvih = idxp.tile([P, nhi // 16], I16, tag="kvih")
            nc.sync.dma_start(out=kvih, in_=kvhi_i[b])
        qib = idxp.tile([P, plan.bcap // 16], I16, tag="qib")
        nc.sync.dma_start(out=qib, in_=q_i[b])

        GMAX = 1024  # per-instruction idx cap (SWDGE descriptor-ring limit)

        def emit_gathers(out_tile, jj_base, table, idx_tile, n_idx, elem):
            for off in range(0, n_idx, GMAX):
                cn = min(GMAX, n_idx - off)
                nc.gpsimd.dma_gather(
                    out_ap=out_tile[:, jj_base + off // P:
                                    jj_base + (off + cn) // P, :],
                    in_ap=table,
                    idxs_ap=idx_tile[:, off // 16:(off + cn) // 16],
                    num_idxs=cn, num_idxs_reg=cn, elem_size=elem)

        kv_blk = gat.tile([P, njj, 2 * hd], F32, tag="kvblk")
        emit_gathers(kv_blk, 0, KVlo[:, :], kvil, nlo, 2 * hd)
        if nhi:
            emit_gathers(kv_blk, slo * cps, KVhi[:, :], kvih, nhi, 2 * hd)
        q_blk = gat.tile([P, njj, hd], F32, tag="qblk")
        emit_gathers(q_blk, 0, Qt[:, :], qib, plan.bcap, hd)

        acc = accp.tile([P, hd + cfg.heads], F32, space="PSUM", tag="acc")

        ea_blk = eap.tile([ind + 1, plan.bcap], F32, tag="ea", bufs=2)
        nc.sync.dma_start(out=ea_blk, in_=ea[b])
        score_blk = outp.tile([P, njj, hd], F32, tag="score", bufs=2)

        for s in range(spb):
            j0 = s * cps

            kv_g = kv_blk[:, j0:j0 + cps, :]
            q_g = q_blk[:, j0:j0 + cps, :]

            e_ps = eps_p.tile([P, cps, hd], F32, space="PSUM", tag="eps")
            for j in range(cps):
                nc.tensor.matmul(out=e_ps[:, j, :],
                                 lhsT=ea_blk[:, (j0 + j) * P:(j0 + j + 1) * P],
                                 rhs=w_eb[:, :], start=True, stop=True)

            kq = work.tile([P, cps, hd], F32, tag="kq")
            nc.vector.tensor_tensor(out=kq[:, :, :], in0=kv_g[:, :, 0:hd],
                                    in1=q_g[:, :, :], op=mybir.AluOpType.mult)
            score = score_blk[:, j0:j0 + cps, :]
            nc.vector.tensor_tensor(out=score[:, :, :], in0=kq[:, :, :],
                                    in1=e_ps[:, :, :], op=mybir.AluOpType.mult)

            sc = work.tile([P, cps, cfg.heads], F32, tag="sc")
            nc.vector.tensor_reduce(
                out=sc[:, :, :],
                in_=score.rearrange("p c (h d) -> p c h d", d=cfg.dhead),
                axis=mybir.AxisListType.X, op=mybir.AluOpType.add)
            nc.vector.tensor_scalar(
                out=sc[:, :, :], in0=sc[:, :, :],
                scalar1=clamp_hi, scalar2=-clamp_hi,
                op0=mybir.AluOpType.min, op1=mybir.AluOpType.max)

            contrib = work.tile([P, cps, hd + cfg.heads], F32, tag="contrib")
            w_ap = contrib[:, :, hd:hd + cfg.heads]
            nc.scalar.activation(out=w_ap, in_=sc[:, :, :],
                                 func=mybir.ActivationFunctionType.Exp,
                                 scale=inv_sqrt_d)

            vps = work.tile([P, cps, hd], F32, tag="vps")
            nc.gpsimd.tensor_tensor(out=vps[:, :, :],
                                    in0=kv_g[:, :, hd:2 * hd],
                                    in1=score[:, :, :],
                                    op=mybir.AluOpType.add)
            w_b = bass.AP(tensor=w_ap.tensor, offset=w_ap.offset,
                          ap=[*w_ap.ap, [0, cfg.dhead]])
            nc.vector.tensor_tensor(
                out=contrib.rearrange("p c (h d) -> p c h d", d=cfg.dhead)[
                    :, :, 0:cfg.heads, :],
                in0=vps.rearrange("p c (h d) -> p c h d", d=cfg.dhead),
                in1=w_b, op=mybir.AluOpType.mult)

            oh = work.tile([P, cps, P], F32, tag="oh")
            loc_s = locb[:, j0:j0 + cps]
            loc_b = bass.AP(tensor=loc_s.tensor, offset=loc_s.offset,
                            ap=[*loc_s.ap, [0, P]])
            it_ap = iota_t[:, :]
            iota_b = bass.AP(tensor=it_ap.tensor, offset=it_ap.offset,
                             ap=[it_ap.ap[0], [0, cps], it_ap.ap[1]])
            nc.vector.tensor_tensor(out=oh[:, :, :], in0=loc_b, in1=iota_b,
                                    op=mybir.AluOpType.is_equal)

            for j in range(cps):
                nc.tensor.matmul(out=acc[:, :], lhsT=oh[:, j, :],
                                 rhs=contrib[:, j, :],
                                 start=(s == 0 and j == 0),
                                 stop=(s == spb - 1 and j == cps - 1))

        nc.sync.dma_start(out=wEs[b], in_=score_blk[:, :, :])

        # --- block finalize: wV = U / (den + 1e-16) ---
        recip = outp.tile([P, cfg.heads], F32, tag="recip")
        nc.vector.tensor_scalar(out=recip, in0=acc[:, hd:hd + cfg.heads],
                                scalar1=1e-16, scalar2=None,
                                op0=mybir.AluOpType.add)
        nc.vector.reciprocal(out=recip, in_=recip)
        wv_t = outp.tile([P, hd], F32, tag="wv")
        r_b = bass.AP(tensor=recip.tensor, offset=recip.offset,
                      ap=[*recip.ap, [0, cfg.dhead]])
        nc.vector.tensor_tensor(
            out=wv_t.rearrange("p (h d) -> p h d", d=cfg.dhead),
            in0=acc[:, 0:hd].rearrange("p (h d) -> p h d", d=cfg.dhead),
            in1=r_b, op=mybir.AluOpType.mult)
        nc.sync.dma_start(out=wVs[b * P:(b + 1) * P, :], in_=wv_t)


def build_program(plan: Plan, repeat: int = 1):
    cfg = plan.cfg
    nc = bacc.Bacc("TRN2", target_bir_lowering=False, debug=False,
                   enable_asserts=False)
    ins = {}
    for name, arr in plan.in_maps[0].items():
        ins[name] = nc.dram_tensor(name, list(arr.shape),
                                   mybir.dt.from_np(arr.dtype),
                                   kind="ExternalInput").ap()
    outs = {
        "wEs": nc.dram_tensor("wEs", [cfg.nb, P, plan.bcap // P * cfg.hd],
                              F32, kind="ExternalOutput").ap(),
        "wVs": nc.dram_tensor("wVs", [cfg.npc_pad, cfg.hd], F32,
                              kind="ExternalOutput").ap(),
    }
    with tile.TileContext(nc) as tc:
        with ExitStack() as ctx:
            build_kernel_body(ctx, tc, outs, ins, plan, repeat=repeat)
    nc.compile()
    return nc


# ---------------------------------------------------------------------------
# top-level entry
# ---------------------------------------------------------------------------


def unshard(plan: Plan, wEs_list, wVs_list):
    cfg = plan.cfg
    u = plan.unshard
    wE = np.empty((u["n_edges"], cfg.hd), np.float32)
    for c in range(cfg.n_cores):
        m = u["core"] == c
        flat = wEs_list[c].reshape(-1, cfg.hd)
        wE[u["ids"][m]] = flat[u["rows"][m]]
    wV = np.concatenate([wVs_list[c][:cfg.npc] for c in range(cfg.n_cores)])
    return wV.reshape(cfg.n_nodes, cfg.heads, cfg.dhead), wE


def kernel(x, edge_attr, WQ, bQ, WK, WV, WE1, bE1, edge_index):
    from concourse.bass_utils import run_bass_kernel_spmd

    cfg = Cfg()
    plan = host_prep(cfg, np.asarray(x, np.float32),
                     np.asarray(edge_attr, np.float32),
                     np.asarray(WQ, np.float32), np.asarray(bQ, np.float32),
                     np.asarray(WK, np.float32), np.asarray(WV, np.float32),
                     np.asarray(WE1, np.float32), np.asarray(bE1, np.float32),
                     np.asarray(edge_index))
    nc = build_program(plan)
    res = run_bass_kernel_spmd(nc, plan.in_maps,
                               core_ids=list(range(cfg.n_cores)))
    wEs = [r["wEs"] for r in res.results]
    wVs = [r["wVs"] for r in res.results]
    return unshard(plan, wEs, wVs)
